# revision 6
# baseline (speedup 1.0000x reference)
"""Trainium2 Bass kernel for the 4-group sparse-tap 3x3 conv.

Computation (see reference): x (32,128,56,56) f32, weights (32,2048) f32.
Four groups of 32 output channels; group g uses 4 taps CFG[g] of the 3x3
footprint over all 128 input channels. Output (32,128,56,56) f32.

Strategy: pure data-parallel over batch — 4 images per NeuronCore, 8 cores.

Host prep: zero-pad each image to 59x58 (1-pixel conv halo + one extra row
so the last shifted matmul view stays in-bounds), cast to fp16, and lay the
4 images of a core out channel-major ([128 ic, 4*59*58]) so shards DMA with
large contiguous per-partition descriptors.  Weights are rearranged into 16
[ic=128, oc=32] fp16 stationary blocks, one per (group, tap) pair.

Device, per image: for each chunk of 8 output rows, issue 16 column-tiled
matmuls (tile_position=(0,32g)) — group g's 4 taps accumulate into PSUM
partitions 32g..32g+31.  Tap (kr,kc) uses the rhs slice starting at
(8c+kr)*58+kc, which yields all 8 shifted rows in one contiguous view
thanks to the width padding.  The 4 groups' matmuls execute concurrently on
the PE's 32-column sub-arrays, so a chunk costs ~4 matmul streams instead
of 9 (the dense-tap formulation): ~785ns/chunk warm.  fp16 keeps 10
mantissa bits (measured end-to-end error ~3e-4) and accumulates in fp32.

DMA plan. HWDGE (sync/scalar rings) generates descriptors serially at
~64ns each, so ANY 128-partition DMA costs ~8.5us on those rings no matter
its size; SWDGE (gpsimd ring) generates in parallel and is transfer-bound.
Therefore: everything latency-critical (weights, first image piece, the
image-3 output pieces at the very end) goes on SWDGE; the three early
whole-image outputs go on the HWDGE rings where their 8.5us hides under
compute.  A few dummy matmuls run during the initial DMA wait to lift the
PE's HAM clock gate (1.2 -> 2.4 GHz) before real work arrives.
"""

from contextlib import ExitStack

import numpy as np

import concourse.bass as bass
import concourse.mybir as mybir
from concourse.bass_utils import run_bass_kernel_spmd

CFG = [[1, 2, 4, 5], [2, 3, 5, 6], [4, 5, 7, 8], [5, 6, 8, 9]]

B, C, H, W = 32, 128, 56, 56
NCORES = 8
BPC = B // NCORES            # images per core
HP, WP = H + 3, W + 2        # padded rows (1 top, 1 bottom, 1 overread), cols
XF = HP * WP                 # 3422 padded free elems per image
OF = H * W                   # 3136 output free elems per image
NPSUM = 8                    # psum banks cycled over chunks
RPC = 8                      # output rows per chunk
NCHUNK = H // RPC            # 7
NFREE = RPC * WP             # 464 matmul free dim
X0A_ROWS = 40                # first piece of image 0 (covers chunks 0-3 + halo)
F32 = mybir.dt.float32
F16 = mybir.dt.float16
SLOT = [0, 1, 2, 0]          # output slot per image (3 slots)


def _build_nc():
    nc = bass.Bass()
    xp = nc.declare_dram_parameter("x", [C, BPC * XF], F16, isOutput=False)
    wp = nc.declare_dram_parameter("w", [C, 16 * 32], F16, isOutput=False)
    op = nc.declare_dram_parameter("out", [BPC, C, OF], F32, isOutput=True)

    with ExitStack() as ctx:
        w_tile = ctx.enter_context(nc.sbuf_tensor("w_tile", [C, 16 * 32], F16))
        xbuf = ctx.enter_context(nc.sbuf_tensor("xbuf", [C, BPC * XF], F16))
        o_slots = [ctx.enter_context(nc.sbuf_tensor(f"o_slot{i}", [C, OF], F32))
                   for i in range(3)]
        psums = [ctx.enter_context(nc.psum_tensor(f"psum{i}", [C, 512], F32))
                 for i in range(NPSUM)]

        w_sem = ctx.enter_context(nc.semaphore("w_sem"))
        x0a_sem = ctx.enter_context(nc.semaphore("x0a_sem"))
        x0b_sem = ctx.enter_context(nc.semaphore("x0b_sem"))
        x1_sem = ctx.enter_context(nc.semaphore("x1_sem"))
        x23_sem = ctx.enter_context(nc.semaphore("x23_sem"))
        mm_sem = ctx.enter_context(nc.semaphore("mm_sem"))
        v_sem = ctx.enter_context(nc.semaphore("v_sem"))
        s_sync = ctx.enter_context(nc.semaphore("s_sync"))
        s_scalar = ctx.enter_context(nc.semaphore("s_scalar"))
        s_gp = ctx.enter_context(nc.semaphore("s_gp"))

        block = ctx.enter_context(nc.Block())

        x0a = X0A_ROWS * WP

        @block.gpsimd
        def _(gpsimd):
            # SWDGE queue: parallel descriptor generation; everything
            # latency-critical lives here.
            gpsimd.dma_start(out=xbuf[:, 0:x0a], in_=xp[:, 0:x0a]).then_inc(x0a_sem, 16)
            gpsimd.dma_start(out=xbuf[:, x0a:XF], in_=xp[:, x0a:XF]).then_inc(x0b_sem, 16)
            gpsimd.dma_start(out=xbuf[:, XF:2 * XF], in_=xp[:, XF:2 * XF]).then_inc(x1_sem, 16)
            gpsimd.dma_start(out=xbuf[:, 2 * XF:], in_=xp[:, 2 * XF:]).then_inc(x23_sem, 16)
            # image-3 output pieces: the kernel tail, SWDGE-only
            gpsimd.wait_ge(v_sem, 3 * NCHUNK + 4)
            gpsimd.dma_start(
                out=op[3][:, :4 * RPC * W],
                in_=o_slots[SLOT[3]][:, :4 * RPC * W],
            ).then_inc(s_gp, 16)
            gpsimd.wait_ge(v_sem, 4 * NCHUNK)
            gpsimd.dma_start(
                out=op[3][:, 4 * RPC * W:],
                in_=o_slots[SLOT[3]][:, 4 * RPC * W:],
            ).then_inc(s_gp, 16)
            gpsimd.wait_ge(s_gp, 32)

        @block.tensor
        def _(tensor):
            # dummy matmuls on garbage data: lift the HAM clock gate while
            # the first image piece is still in flight (bank 7 is clobbered
            # by real work much later).
            for _ in range(10):
                tensor.matmul(
                    psums[NPSUM - 1][0:32, :NFREE],
                    w_tile[:, 0:32],
                    xbuf[:, 0:NFREE],
                    start=True, stop=True,
                    tile_position=(0, 0),
                )
            tensor.wait_ge(w_sem, 16)
            tensor.wait_ge(x0a_sem, 16)
            for b in range(BPC):
                for c in range(NCHUNK):
                    g = NCHUNK * b + c
                    if g == 4:
                        tensor.wait_ge(x0b_sem, 16)
                    elif g == NCHUNK:
                        tensor.wait_ge(x1_sem, 16)
                    elif g == 2 * NCHUNK:
                        tensor.wait_ge(x23_sem, 16)
                    if g >= NPSUM:
                        # psum bank g%NPSUM free once chunk g-NPSUM was copied
                        tensor.wait_ge(v_sem, g - NPSUM + 1)
                    bank = psums[g % NPSUM]
                    for j in range(4):
                        for grp in range(4):
                            t = CFG[grp][j]
                            kr, kc = (t - 1) // 3, (t - 1) % 3
                            off = b * XF + (RPC * c + kr) * WP + kc
                            idx = grp * 4 + j
                            mm = tensor.matmul(
                                bank[32 * grp:32 * (grp + 1), :NFREE],
                                w_tile[:, idx * 32:(idx + 1) * 32],
                                xbuf[:, off:off + NFREE],
                                start=(j == 0),
                                stop=(j == 3),
                                tile_position=(0, 32 * grp),
                            )
                    mm.then_inc(mm_sem, 1)

        @block.vector
        def _(vector):
            for b in range(BPC):
                if b == 3:
                    vector.wait_ge(s_sync, 16)   # out0 done -> slot0 free
                for c in range(NCHUNK):
                    g = NCHUNK * b + c
                    vector.wait_ge(mm_sem, g + 1)
                    src = psums[g % NPSUM][:, :NFREE].rearrange(
                        "p (r w) -> p r w", w=WP)[:, :, :W]
                    dst = o_slots[SLOT[b]][:, c * RPC * W:(c + 1) * RPC * W].rearrange(
                        "p (r w) -> p r w", w=W)
                    vector.tensor_copy(out=dst, in_=src).then_inc(v_sem, 1)

        @block.sync
        def _(sync):
            sync.dma_start(out=w_tile[:], in_=wp[:]).then_inc(w_sem, 16)
            sync.wait_ge(v_sem, NCHUNK)
            sync.dma_start(out=op[0], in_=o_slots[SLOT[0]][:]).then_inc(s_sync, 16)
            sync.wait_ge(v_sem, 3 * NCHUNK)
            sync.dma_start(out=op[2], in_=o_slots[SLOT[2]][:]).then_inc(s_sync, 16)
            sync.wait_ge(s_sync, 32)

        @block.scalar
        def _(scalar):
            scalar.wait_ge(v_sem, 2 * NCHUNK)
            scalar.dma_start(out=op[1], in_=o_slots[SLOT[1]][:]).then_inc(s_scalar, 16)
            scalar.wait_ge(s_scalar, 16)

    return nc


_NC_CACHE = None


def _get_nc():
    global _NC_CACHE
    if _NC_CACHE is None:
        _NC_CACHE = _build_nc()
    return _NC_CACHE


def _prep_weights(weights):
    """(32, 2048) grouped-sparse -> 16 [ic=128, oc=32] fp16 lhsT blocks."""
    w16 = np.zeros((C, 16 * 32), np.float32)
    for g, taps in enumerate(CFG):
        blk = np.asarray(weights[:, g * 512:(g + 1) * 512], np.float32)
        blk = blk.reshape(32, C, 4)  # [oc_in_group, ic, tap_j]
        for j in range(4):
            idx = g * 4 + j
            w16[:, idx * 32:(idx + 1) * 32] = blk[:, :, j].T
    return np.ascontiguousarray(w16.astype(np.float16))


def _prep_x(x):
    """(32,128,56,56) f32 -> per-core channel-major padded fp16 shards."""
    xpad = np.zeros((B, C, HP, WP), np.float16)
    xpad[:, :, 1:H + 1, 1:W + 1] = x.astype(np.float16)
    xs = xpad.reshape(NCORES, BPC, C, XF)
    # (core, b, c, f) -> (core, c, b*f)
    xs = np.ascontiguousarray(xs.transpose(0, 2, 1, 3)).reshape(NCORES, C, BPC * XF)
    return xs


def kernel(x, weights):
    x = np.asarray(x, np.float32)
    weights = np.asarray(weights, np.float32)

    xs = _prep_x(x)
    wflat = _prep_weights(weights)

    nc = _get_nc()
    in_maps = [{"x": xs[i], "w": wflat} for i in range(NCORES)]
    res = run_bass_kernel_spmd(nc, in_maps, core_ids=list(range(NCORES)))
    return np.concatenate(
        [res.results[i]["out"].reshape(BPC, C, H, W) for i in range(NCORES)],
        axis=0,
    )


# revision 7
# speedup vs baseline: 1.0154x; 1.0154x over previous
"""Trainium2 Bass kernel for the 4-group sparse-tap 3x3 conv.

Computation (see reference): x (32,128,56,56) f32, weights (32,2048) f32.
Four groups of 32 output channels; group g uses 4 taps CFG[g] of the 3x3
footprint over all 128 input channels. Output (32,128,56,56) f32.

Strategy: pure data-parallel over batch — 4 images per NeuronCore, 8 cores.

Host prep: zero-pad each image to 59x58 (1-pixel conv halo + one extra row
so the last shifted matmul view stays in-bounds), cast to fp16, and lay the
4 images of a core out channel-major ([128 ic, 4*59*58]) so shards DMA with
large contiguous per-partition descriptors.  Weights are rearranged into 16
[ic=128, oc=32] fp16 stationary blocks, one per (group, tap) pair.

Device, per image: for each chunk of 8 output rows, issue 16 column-tiled
matmuls (tile_position=(0,32g)) — group g's 4 taps accumulate into PSUM
partitions 32g..32g+31.  Tap (kr,kc) uses the rhs slice starting at
(8c+kr)*58+kc, which yields all 8 shifted rows in one contiguous view
thanks to the width padding.  The 4 groups' matmuls execute concurrently on
the PE's 32-column sub-arrays, so a chunk costs ~4 matmul streams instead
of 9 (the dense-tap formulation): ~785ns/chunk warm.  fp16 keeps 10
mantissa bits (measured end-to-end error ~3e-4) and accumulates in fp32.

DMA plan. HWDGE (sync/scalar rings) generates descriptors serially at
~64ns each, so ANY 128-partition DMA costs ~8.5us on those rings no matter
its size; SWDGE (gpsimd ring) generates in parallel and is transfer-bound.
Therefore: everything latency-critical (weights, first image piece, the
image-3 output pieces at the very end) goes on SWDGE; the three early
whole-image outputs go on the HWDGE rings where their 8.5us hides under
compute.  A few dummy matmuls run during the initial DMA wait to lift the
PE's HAM clock gate (1.2 -> 2.4 GHz) before real work arrives.
"""

from contextlib import ExitStack

import numpy as np

import concourse.bass as bass
import concourse.mybir as mybir
from concourse.bass_utils import run_bass_kernel_spmd

CFG = [[1, 2, 4, 5], [2, 3, 5, 6], [4, 5, 7, 8], [5, 6, 8, 9]]

B, C, H, W = 32, 128, 56, 56
NCORES = 8
BPC = B // NCORES            # images per core
HP, WP = H + 3, W + 2        # padded rows (1 top, 1 bottom, 1 overread), cols
XF = HP * WP                 # 3422 padded free elems per image
OF = H * W                   # 3136 output free elems per image
NPSUM = 8                    # psum banks cycled over chunks
RPC = 8                      # output rows per chunk
NCHUNK = H // RPC            # 7
NFREE = RPC * WP             # 464 matmul free dim
X0A_ROWS = 40                # first piece of image 0 (covers chunks 0-3 + halo)
F32 = mybir.dt.float32
F16 = mybir.dt.float16
SLOT = [0, 1, 2, 0]          # output slot per image (3 slots)


def _build_nc():
    nc = bass.Bass()
    xp = nc.declare_dram_parameter("x", [C, BPC * XF], F16, isOutput=False)
    wp = nc.declare_dram_parameter("w", [C, 16 * 32], F16, isOutput=False)
    op = nc.declare_dram_parameter("out", [BPC, C, OF], F32, isOutput=True)

    with ExitStack() as ctx:
        w_tile = ctx.enter_context(nc.sbuf_tensor("w_tile", [C, 16 * 32], F16))
        xbuf = ctx.enter_context(nc.sbuf_tensor("xbuf", [C, BPC * XF], F16))
        o_slots = [ctx.enter_context(nc.sbuf_tensor(f"o_slot{i}", [C, OF], F32))
                   for i in range(3)]
        psums = [ctx.enter_context(nc.psum_tensor(f"psum{i}", [C, 512], F32))
                 for i in range(NPSUM)]

        w_sem = ctx.enter_context(nc.semaphore("w_sem"))
        x0a_sem = ctx.enter_context(nc.semaphore("x0a_sem"))
        x0b_sem = ctx.enter_context(nc.semaphore("x0b_sem"))
        x1_sem = ctx.enter_context(nc.semaphore("x1_sem"))
        x23_sem = ctx.enter_context(nc.semaphore("x23_sem"))
        mm_sem = ctx.enter_context(nc.semaphore("mm_sem"))
        v_sem = ctx.enter_context(nc.semaphore("v_sem"))
        s_sync = ctx.enter_context(nc.semaphore("s_sync"))
        s_scalar = ctx.enter_context(nc.semaphore("s_scalar"))
        s_gp = ctx.enter_context(nc.semaphore("s_gp"))

        block = ctx.enter_context(nc.Block())

        x0a = X0A_ROWS * WP

        @block.gpsimd
        def _(gpsimd):
            # SWDGE queue: parallel descriptor generation; everything
            # latency-critical lives here.
            gpsimd.dma_start(out=xbuf[:, 0:x0a], in_=xp[:, 0:x0a]).then_inc(x0a_sem, 16)
            gpsimd.dma_start(out=xbuf[:, x0a:XF], in_=xp[:, x0a:XF]).then_inc(x0b_sem, 16)
            gpsimd.dma_start(out=xbuf[:, XF:2 * XF], in_=xp[:, XF:2 * XF]).then_inc(x1_sem, 16)
            gpsimd.dma_start(out=xbuf[:, 2 * XF:], in_=xp[:, 2 * XF:]).then_inc(x23_sem, 16)
            # image-3 tail piece: SWDGE so no descriptor-gen latency at the end
            gpsimd.wait_ge(v_sem, 4 * NCHUNK)
            gpsimd.dma_start(
                out=op[3][:, 4 * RPC * W:],
                in_=o_slots[SLOT[3]][:, 4 * RPC * W:],
            ).then_inc(s_gp, 16)
            gpsimd.wait_ge(s_gp, 16)

        @block.tensor
        def _(tensor):
            # dummy matmuls on garbage data: lift the HAM clock gate while
            # the first image piece is still in flight (bank 7 is clobbered
            # by real work much later).
            for _ in range(13):
                tensor.matmul(
                    psums[NPSUM - 1][0:32, :NFREE],
                    w_tile[:, 0:32],
                    xbuf[:, 0:NFREE],
                    start=True, stop=True,
                    tile_position=(0, 0),
                )
            tensor.wait_ge(w_sem, 16)
            tensor.wait_ge(x0a_sem, 16)
            for b in range(BPC):
                for c in range(NCHUNK):
                    g = NCHUNK * b + c
                    if g == 4:
                        tensor.wait_ge(x0b_sem, 16)
                    elif g == NCHUNK:
                        tensor.wait_ge(x1_sem, 16)
                    elif g == 2 * NCHUNK:
                        tensor.wait_ge(x23_sem, 16)
                    if g >= NPSUM:
                        # psum bank g%NPSUM free once chunk g-NPSUM was copied
                        tensor.wait_ge(v_sem, g - NPSUM + 1)
                    bank = psums[g % NPSUM]
                    for j in range(4):
                        for grp in range(4):
                            t = CFG[grp][j]
                            kr, kc = (t - 1) // 3, (t - 1) % 3
                            off = b * XF + (RPC * c + kr) * WP + kc
                            idx = grp * 4 + j
                            mm = tensor.matmul(
                                bank[32 * grp:32 * (grp + 1), :NFREE],
                                w_tile[:, idx * 32:(idx + 1) * 32],
                                xbuf[:, off:off + NFREE],
                                start=(j == 0),
                                stop=(j == 3),
                                tile_position=(0, 32 * grp),
                            )
                    mm.then_inc(mm_sem, 1)

        @block.vector
        def _(vector):
            for b in range(BPC):
                if b == 3:
                    vector.wait_ge(s_sync, 16)   # out0 done -> slot0 free
                for c in range(NCHUNK):
                    g = NCHUNK * b + c
                    vector.wait_ge(mm_sem, g + 1)
                    src = psums[g % NPSUM][:, :NFREE].rearrange(
                        "p (r w) -> p r w", w=WP)[:, :, :W]
                    dst = o_slots[SLOT[b]][:, c * RPC * W:(c + 1) * RPC * W].rearrange(
                        "p (r w) -> p r w", w=W)
                    vector.tensor_copy(out=dst, in_=src).then_inc(v_sem, 1)

        @block.sync
        def _(sync):
            sync.dma_start(out=w_tile[:], in_=wp[:]).then_inc(w_sem, 16)
            sync.wait_ge(v_sem, NCHUNK)
            sync.dma_start(out=op[0], in_=o_slots[SLOT[0]][:]).then_inc(s_sync, 16)
            sync.wait_ge(v_sem, 3 * NCHUNK)
            sync.dma_start(out=op[2], in_=o_slots[SLOT[2]][:]).then_inc(s_sync, 16)
            sync.wait_ge(s_sync, 32)

        @block.scalar
        def _(scalar):
            scalar.wait_ge(v_sem, 2 * NCHUNK)
            scalar.dma_start(out=op[1], in_=o_slots[SLOT[1]][:]).then_inc(s_scalar, 16)
            # first 4 chunks of image 3 as soon as they are copied
            scalar.wait_ge(v_sem, 3 * NCHUNK + 4)
            scalar.dma_start(
                out=op[3][:, :4 * RPC * W],
                in_=o_slots[SLOT[3]][:, :4 * RPC * W],
            ).then_inc(s_scalar, 16)
            scalar.wait_ge(s_scalar, 32)

    return nc


_NC_CACHE = None


def _get_nc():
    global _NC_CACHE
    if _NC_CACHE is None:
        _NC_CACHE = _build_nc()
    return _NC_CACHE


def _prep_weights(weights):
    """(32, 2048) grouped-sparse -> 16 [ic=128, oc=32] fp16 lhsT blocks."""
    w16 = np.zeros((C, 16 * 32), np.float32)
    for g, taps in enumerate(CFG):
        blk = np.asarray(weights[:, g * 512:(g + 1) * 512], np.float32)
        blk = blk.reshape(32, C, 4)  # [oc_in_group, ic, tap_j]
        for j in range(4):
            idx = g * 4 + j
            w16[:, idx * 32:(idx + 1) * 32] = blk[:, :, j].T
    return np.ascontiguousarray(w16.astype(np.float16))


def _prep_x(x):
    """(32,128,56,56) f32 -> per-core channel-major padded fp16 shards."""
    xpad = np.zeros((B, C, HP, WP), np.float16)
    xpad[:, :, 1:H + 1, 1:W + 1] = x.astype(np.float16)
    xs = xpad.reshape(NCORES, BPC, C, XF)
    # (core, b, c, f) -> (core, c, b*f)
    xs = np.ascontiguousarray(xs.transpose(0, 2, 1, 3)).reshape(NCORES, C, BPC * XF)
    return xs


def kernel(x, weights):
    x = np.asarray(x, np.float32)
    weights = np.asarray(weights, np.float32)

    xs = _prep_x(x)
    wflat = _prep_weights(weights)

    nc = _get_nc()
    in_maps = [{"x": xs[i], "w": wflat} for i in range(NCORES)]
    res = run_bass_kernel_spmd(nc, in_maps, core_ids=list(range(NCORES)))
    return np.concatenate(
        [res.results[i]["out"].reshape(BPC, C, H, W) for i in range(NCORES)],
        axis=0,
    )


# revision 8
# speedup vs baseline: 1.0841x; 1.0677x over previous
"""Trainium2 Bass kernel for the 4-group sparse-tap 3x3 conv.

Computation (see reference): x (32,128,56,56) f32, weights (32,2048) f32.
Four groups of 32 output channels; group g uses 4 taps CFG[g] of the 3x3
footprint over all 128 input channels. Output (32,128,56,56) f32.

Strategy: pure data-parallel over batch — 4 images per NeuronCore, 8 cores.

Host prep: zero-pad each image to 59x58 (1-pixel conv halo + one extra row
so the last shifted matmul view stays in-bounds), cast to fp16, and lay the
4 images of a core out channel-major ([128 ic, 4*59*58]) so shards DMA with
large contiguous per-partition descriptors.  Weights are rearranged into 16
[ic=128, oc=32] fp16 stationary blocks, one per (group, tap) pair.

Device, per image: for each chunk of 8 output rows, issue 16 column-tiled
matmuls (tile_position=(0,32g)) — group g's 4 taps accumulate into PSUM
partitions 32g..32g+31.  Tap (kr,kc) uses the rhs slice starting at
(8c+kr)*58+kc, which yields all 8 shifted rows in one contiguous view
thanks to the width padding.  The 4 groups' matmuls execute concurrently on
the PE's 32-column sub-arrays, so a chunk costs ~4 matmul streams instead
of 9 (the dense-tap formulation): ~785ns/chunk warm.  fp16 keeps 10
mantissa bits (measured end-to-end error ~3e-4) and accumulates in fp32.

DMA plan. HWDGE (sync/scalar rings) generates descriptors serially at
~64ns each, so ANY 128-partition DMA costs ~8.5us on those rings no matter
its size; SWDGE (gpsimd ring) generates in parallel and is transfer-bound.
Therefore: everything latency-critical (weights, first image piece, the
image-3 output pieces at the very end) goes on SWDGE; the three early
whole-image outputs go on the HWDGE rings where their 8.5us hides under
compute.  A few dummy matmuls run during the initial DMA wait to lift the
PE's HAM clock gate (1.2 -> 2.4 GHz) before real work arrives.
"""

from contextlib import ExitStack

import numpy as np

import concourse.bass as bass
import concourse.mybir as mybir
from concourse.bass_utils import run_bass_kernel_spmd

CFG = [[1, 2, 4, 5], [2, 3, 5, 6], [4, 5, 7, 8], [5, 6, 8, 9]]

B, C, H, W = 32, 128, 56, 56
NCORES = 8
BPC = B // NCORES            # images per core
HP, WP = H + 3, W + 2        # padded rows (1 top, 1 bottom, 1 overread), cols
XF = HP * WP                 # 3422 padded free elems per image
OF = H * W                   # 3136 output free elems per image
NPSUM = 8                    # psum banks cycled over chunks
RPC = 8                      # output rows per chunk
NCHUNK = H // RPC            # 7
NFREE = RPC * WP             # 464 matmul free dim
X0A_END = 18 * WP            # rows 0-17: chunks 0-1
X0B_END = 34 * WP            # rows 18-33: chunks 2-3
F32 = mybir.dt.float32
F16 = mybir.dt.float16
SLOT = [0, 1, 2, 0]          # output slot per image (3 slots)


def _build_nc():
    nc = bass.Bass()
    xp = nc.declare_dram_parameter("x", [C, BPC * XF], F16, isOutput=False)
    wp = nc.declare_dram_parameter("w", [C, 16 * 32], F16, isOutput=False)
    op = nc.declare_dram_parameter("out", [BPC, C, OF], F32, isOutput=True)

    with ExitStack() as ctx:
        w_tile = ctx.enter_context(nc.sbuf_tensor("w_tile", [C, 16 * 32], F16))
        xbuf = ctx.enter_context(nc.sbuf_tensor("xbuf", [C, BPC * XF], F16))
        o_slots = [ctx.enter_context(nc.sbuf_tensor(f"o_slot{i}", [C, OF], F32))
                   for i in range(3)]
        psums = [ctx.enter_context(nc.psum_tensor(f"psum{i}", [C, 512], F32))
                 for i in range(NPSUM)]

        w_sem = ctx.enter_context(nc.semaphore("w_sem"))
        x0a_sem = ctx.enter_context(nc.semaphore("x0a_sem"))
        x0b_sem = ctx.enter_context(nc.semaphore("x0b_sem"))
        x0c_sem = ctx.enter_context(nc.semaphore("x0c_sem"))
        x1_sem = ctx.enter_context(nc.semaphore("x1_sem"))
        x23_sem = ctx.enter_context(nc.semaphore("x23_sem"))
        mm_sem = ctx.enter_context(nc.semaphore("mm_sem"))
        v_sem = ctx.enter_context(nc.semaphore("v_sem"))
        s_sync = ctx.enter_context(nc.semaphore("s_sync"))
        s_scalar = ctx.enter_context(nc.semaphore("s_scalar"))
        s_gp = ctx.enter_context(nc.semaphore("s_gp"))

        block = ctx.enter_context(nc.Block())

        @block.gpsimd
        def _(gpsimd):
            # SWDGE queue: parallel descriptor generation; everything
            # latency-critical lives here.
            gpsimd.dma_start(out=xbuf[:, 0:X0A_END], in_=xp[:, 0:X0A_END]).then_inc(x0a_sem, 16)
            gpsimd.dma_start(out=xbuf[:, X0A_END:X0B_END], in_=xp[:, X0A_END:X0B_END]).then_inc(x0b_sem, 16)
            gpsimd.dma_start(out=xbuf[:, X0B_END:XF], in_=xp[:, X0B_END:XF]).then_inc(x0c_sem, 16)
            gpsimd.dma_start(out=xbuf[:, XF:2 * XF], in_=xp[:, XF:2 * XF]).then_inc(x1_sem, 16)
            gpsimd.dma_start(out=xbuf[:, 2 * XF:], in_=xp[:, 2 * XF:]).then_inc(x23_sem, 16)
            # image-3 tail piece: SWDGE so no descriptor-gen latency at the end
            gpsimd.wait_ge(v_sem, 4 * NCHUNK)
            gpsimd.dma_start(
                out=op[3][:, 4 * RPC * W:],
                in_=o_slots[SLOT[3]][:, 4 * RPC * W:],
            ).then_inc(s_gp, 16)
            gpsimd.wait_ge(s_gp, 16)

        @block.tensor
        def _(tensor):
            # dummy matmuls on garbage data: lift the HAM clock gate while
            # the first image piece is still in flight (bank 7 is clobbered
            # by real work much later).
            for _ in range(9):
                tensor.matmul(
                    psums[NPSUM - 1][0:32, :NFREE],
                    w_tile[:, 0:32],
                    xbuf[:, 0:NFREE],
                    start=True, stop=True,
                    tile_position=(0, 0),
                )
            tensor.wait_ge(w_sem, 16)
            tensor.wait_ge(x0a_sem, 16)
            for b in range(BPC):
                for c in range(NCHUNK):
                    g = NCHUNK * b + c
                    if g == 2:
                        tensor.wait_ge(x0b_sem, 16)
                    elif g == 4:
                        tensor.wait_ge(x0c_sem, 16)
                    elif g == NCHUNK:
                        tensor.wait_ge(x1_sem, 16)
                    elif g == 2 * NCHUNK:
                        tensor.wait_ge(x23_sem, 16)
                    if g >= NPSUM:
                        # psum bank g%NPSUM free once chunk g-NPSUM was copied
                        tensor.wait_ge(v_sem, g - NPSUM + 1)
                    bank = psums[g % NPSUM]
                    for j in range(4):
                        for grp in range(4):
                            t = CFG[grp][j]
                            kr, kc = (t - 1) // 3, (t - 1) % 3
                            off = b * XF + (RPC * c + kr) * WP + kc
                            idx = grp * 4 + j
                            mm = tensor.matmul(
                                bank[32 * grp:32 * (grp + 1), :NFREE],
                                w_tile[:, idx * 32:(idx + 1) * 32],
                                xbuf[:, off:off + NFREE],
                                start=(j == 0),
                                stop=(j == 3),
                                tile_position=(0, 32 * grp),
                            )
                    mm.then_inc(mm_sem, 1)

        @block.vector
        def _(vector):
            for b in range(BPC):
                if b == 3:
                    vector.wait_ge(s_sync, 16)   # out0 done -> slot0 free
                for c in range(NCHUNK):
                    g = NCHUNK * b + c
                    vector.wait_ge(mm_sem, g + 1)
                    src = psums[g % NPSUM][:, :NFREE].rearrange(
                        "p (r w) -> p r w", w=WP)[:, :, :W]
                    dst = o_slots[SLOT[b]][:, c * RPC * W:(c + 1) * RPC * W].rearrange(
                        "p (r w) -> p r w", w=W)
                    vector.tensor_copy(out=dst, in_=src).then_inc(v_sem, 1)

        @block.sync
        def _(sync):
            sync.dma_start(out=w_tile[:], in_=wp[:]).then_inc(w_sem, 16)
            sync.wait_ge(v_sem, NCHUNK)
            sync.dma_start(out=op[0], in_=o_slots[SLOT[0]][:]).then_inc(s_sync, 16)
            sync.wait_ge(v_sem, 3 * NCHUNK)
            sync.dma_start(out=op[2], in_=o_slots[SLOT[2]][:]).then_inc(s_sync, 16)
            sync.wait_ge(s_sync, 32)

        @block.scalar
        def _(scalar):
            scalar.wait_ge(v_sem, 2 * NCHUNK)
            scalar.dma_start(out=op[1], in_=o_slots[SLOT[1]][:]).then_inc(s_scalar, 16)
            # first 4 chunks of image 3 as soon as they are copied
            scalar.wait_ge(v_sem, 3 * NCHUNK + 4)
            scalar.dma_start(
                out=op[3][:, :4 * RPC * W],
                in_=o_slots[SLOT[3]][:, :4 * RPC * W],
            ).then_inc(s_scalar, 16)
            scalar.wait_ge(s_scalar, 32)

    return nc


_NC_CACHE = None


def _get_nc():
    global _NC_CACHE
    if _NC_CACHE is None:
        _NC_CACHE = _build_nc()
    return _NC_CACHE


def _prep_weights(weights):
    """(32, 2048) grouped-sparse -> 16 [ic=128, oc=32] fp16 lhsT blocks."""
    w16 = np.zeros((C, 16 * 32), np.float32)
    for g, taps in enumerate(CFG):
        blk = np.asarray(weights[:, g * 512:(g + 1) * 512], np.float32)
        blk = blk.reshape(32, C, 4)  # [oc_in_group, ic, tap_j]
        for j in range(4):
            idx = g * 4 + j
            w16[:, idx * 32:(idx + 1) * 32] = blk[:, :, j].T
    return np.ascontiguousarray(w16.astype(np.float16))


def _prep_x(x):
    """(32,128,56,56) f32 -> per-core channel-major padded fp16 shards."""
    xpad = np.zeros((B, C, HP, WP), np.float16)
    xpad[:, :, 1:H + 1, 1:W + 1] = x.astype(np.float16)
    xs = xpad.reshape(NCORES, BPC, C, XF)
    # (core, b, c, f) -> (core, c, b*f)
    xs = np.ascontiguousarray(xs.transpose(0, 2, 1, 3)).reshape(NCORES, C, BPC * XF)
    return xs


def kernel(x, weights):
    x = np.asarray(x, np.float32)
    weights = np.asarray(weights, np.float32)

    xs = _prep_x(x)
    wflat = _prep_weights(weights)

    nc = _get_nc()
    in_maps = [{"x": xs[i], "w": wflat} for i in range(NCORES)]
    res = run_bass_kernel_spmd(nc, in_maps, core_ids=list(range(NCORES)))
    return np.concatenate(
        [res.results[i]["out"].reshape(BPC, C, H, W) for i in range(NCORES)],
        axis=0,
    )


# revision 9
# speedup vs baseline: 1.0897x; 1.0052x over previous
"""Trainium2 Bass kernel for the 4-group sparse-tap 3x3 conv.

Computation (see reference): x (32,128,56,56) f32, weights (32,2048) f32.
Four groups of 32 output channels; group g uses 4 taps CFG[g] of the 3x3
footprint over all 128 input channels. Output (32,128,56,56) f32.

Strategy: pure data-parallel over batch — 4 images per NeuronCore, 8 cores.

Host prep: zero-pad each image to 59x58 (1-pixel conv halo + one extra row
so the last shifted matmul view stays in-bounds), cast to fp16, and lay the
4 images of a core out channel-major ([128 ic, 4*59*58]) so shards DMA with
large contiguous per-partition descriptors.  Weights are rearranged into 16
[ic=128, oc=32] fp16 stationary blocks, one per (group, tap) pair.

Device, per image: for each chunk of 8 output rows, issue 16 column-tiled
matmuls (tile_position=(0,32g)) — group g's 4 taps accumulate into PSUM
partitions 32g..32g+31.  Tap (kr,kc) uses the rhs slice starting at
(8c+kr)*58+kc, which yields all 8 shifted rows in one contiguous view
thanks to the width padding.  The 4 groups' matmuls execute concurrently on
the PE's 32-column sub-arrays, so a chunk costs ~4 matmul streams instead
of 9 (the dense-tap formulation): ~785ns/chunk warm.  fp16 keeps 10
mantissa bits (measured end-to-end error ~3e-4) and accumulates in fp32.

DMA plan. HWDGE (sync/scalar rings) generates descriptors serially at
~64ns each, so ANY 128-partition DMA costs ~8.5us on those rings no matter
its size; SWDGE (gpsimd ring) generates in parallel and is transfer-bound.
Therefore: everything latency-critical (weights, first image piece, the
image-3 output pieces at the very end) goes on SWDGE; the three early
whole-image outputs go on the HWDGE rings where their 8.5us hides under
compute.  A few dummy matmuls run during the initial DMA wait to lift the
PE's HAM clock gate (1.2 -> 2.4 GHz) before real work arrives.
"""

from contextlib import ExitStack

import numpy as np

import concourse.bass as bass
import concourse.mybir as mybir
from concourse.bass_utils import run_bass_kernel_spmd

CFG = [[1, 2, 4, 5], [2, 3, 5, 6], [4, 5, 7, 8], [5, 6, 8, 9]]

B, C, H, W = 32, 128, 56, 56
NCORES = 8
BPC = B // NCORES            # images per core
HP, WP = H + 3, W + 2        # padded rows (1 top, 1 bottom, 1 overread), cols
XF = HP * WP                 # 3422 padded free elems per image
OF = H * W                   # 3136 output free elems per image
NPSUM = 8                    # psum banks cycled over chunks
RPC = 8                      # output rows per chunk
NCHUNK = H // RPC            # 7
NFREE = RPC * WP             # 464 matmul free dim
X0A_END = 18 * WP            # rows 0-17: chunks 0-1
X0B_END = 34 * WP            # rows 18-33: chunks 2-3
F32 = mybir.dt.float32
F16 = mybir.dt.float16
SLOT = [0, 1, 2, 0]          # output slot per image (3 slots)


def _build_nc():
    nc = bass.Bass()
    xp = nc.declare_dram_parameter("x", [C, BPC * XF], F16, isOutput=False)
    wp = nc.declare_dram_parameter("w", [C, 16 * 32], F16, isOutput=False)
    op = nc.declare_dram_parameter("out", [BPC, C, OF], F32, isOutput=True)

    with ExitStack() as ctx:
        w_tile = ctx.enter_context(nc.sbuf_tensor("w_tile", [C, 16 * 32], F16))
        xbuf = ctx.enter_context(nc.sbuf_tensor("xbuf", [C, BPC * XF], F16))
        o_slots = [ctx.enter_context(nc.sbuf_tensor(f"o_slot{i}", [C, OF], F32))
                   for i in range(3)]
        psums = [ctx.enter_context(nc.psum_tensor(f"psum{i}", [C, 512], F32))
                 for i in range(NPSUM)]

        w_sem = ctx.enter_context(nc.semaphore("w_sem"))
        x0a_sem = ctx.enter_context(nc.semaphore("x0a_sem"))
        x0b_sem = ctx.enter_context(nc.semaphore("x0b_sem"))
        x0c_sem = ctx.enter_context(nc.semaphore("x0c_sem"))
        x1_sem = ctx.enter_context(nc.semaphore("x1_sem"))
        x23_sem = ctx.enter_context(nc.semaphore("x23_sem"))
        mm_sem = ctx.enter_context(nc.semaphore("mm_sem"))
        v_sem = ctx.enter_context(nc.semaphore("v_sem"))
        s_sync = ctx.enter_context(nc.semaphore("s_sync"))
        s_scalar = ctx.enter_context(nc.semaphore("s_scalar"))
        s_gp = ctx.enter_context(nc.semaphore("s_gp"))

        block = ctx.enter_context(nc.Block())

        @block.gpsimd
        def _(gpsimd):
            # SWDGE queue: parallel descriptor generation; everything
            # latency-critical lives here.
            gpsimd.dma_start(out=xbuf[:, 0:X0A_END], in_=xp[:, 0:X0A_END]).then_inc(x0a_sem, 16)
            gpsimd.dma_start(out=xbuf[:, X0A_END:X0B_END], in_=xp[:, X0A_END:X0B_END]).then_inc(x0b_sem, 16)
            gpsimd.dma_start(out=xbuf[:, X0B_END:XF], in_=xp[:, X0B_END:XF]).then_inc(x0c_sem, 16)
            gpsimd.dma_start(out=xbuf[:, XF:2 * XF], in_=xp[:, XF:2 * XF]).then_inc(x1_sem, 16)
            gpsimd.dma_start(out=xbuf[:, 2 * XF:], in_=xp[:, 2 * XF:]).then_inc(x23_sem, 16)
            # image-3 tail pieces: SWDGE so no descriptor-gen latency at the end
            gpsimd.wait_ge(v_sem, 3 * NCHUNK + 5)
            gpsimd.dma_start(
                out=op[3][:, 3 * RPC * W:5 * RPC * W],
                in_=o_slots[SLOT[3]][:, 3 * RPC * W:5 * RPC * W],
            ).then_inc(s_gp, 16)
            gpsimd.wait_ge(v_sem, 4 * NCHUNK)
            gpsimd.dma_start(
                out=op[3][:, 5 * RPC * W:],
                in_=o_slots[SLOT[3]][:, 5 * RPC * W:],
            ).then_inc(s_gp, 16)
            gpsimd.wait_ge(s_gp, 32)

        @block.tensor
        def _(tensor):
            # dummy matmuls on garbage data: lift the HAM clock gate while
            # the first image piece is still in flight (bank 7 is clobbered
            # by real work much later).
            for _ in range(9):
                tensor.matmul(
                    psums[NPSUM - 1][0:32, :NFREE],
                    w_tile[:, 0:32],
                    xbuf[:, 0:NFREE],
                    start=True, stop=True,
                    tile_position=(0, 0),
                )
            tensor.wait_ge(w_sem, 16)
            tensor.wait_ge(x0a_sem, 16)
            for b in range(BPC):
                for c in range(NCHUNK):
                    g = NCHUNK * b + c
                    if g == 2:
                        tensor.wait_ge(x0b_sem, 16)
                    elif g == 4:
                        tensor.wait_ge(x0c_sem, 16)
                    elif g == NCHUNK:
                        tensor.wait_ge(x1_sem, 16)
                    elif g == 2 * NCHUNK:
                        tensor.wait_ge(x23_sem, 16)
                    if g >= NPSUM:
                        # psum bank g%NPSUM free once chunk g-NPSUM was copied
                        tensor.wait_ge(v_sem, g - NPSUM + 1)
                    bank = psums[g % NPSUM]
                    for j in range(4):
                        for grp in range(4):
                            t = CFG[grp][j]
                            kr, kc = (t - 1) // 3, (t - 1) % 3
                            off = b * XF + (RPC * c + kr) * WP + kc
                            idx = grp * 4 + j
                            mm = tensor.matmul(
                                bank[32 * grp:32 * (grp + 1), :NFREE],
                                w_tile[:, idx * 32:(idx + 1) * 32],
                                xbuf[:, off:off + NFREE],
                                start=(j == 0),
                                stop=(j == 3),
                                tile_position=(0, 32 * grp),
                            )
                    mm.then_inc(mm_sem, 1)

        @block.vector
        def _(vector):
            for b in range(BPC):
                if b == 3:
                    vector.wait_ge(s_sync, 16)   # out0 done -> slot0 free
                for c in range(NCHUNK):
                    g = NCHUNK * b + c
                    vector.wait_ge(mm_sem, g + 1)
                    src = psums[g % NPSUM][:, :NFREE].rearrange(
                        "p (r w) -> p r w", w=WP)[:, :, :W]
                    dst = o_slots[SLOT[b]][:, c * RPC * W:(c + 1) * RPC * W].rearrange(
                        "p (r w) -> p r w", w=W)
                    vector.tensor_copy(out=dst, in_=src).then_inc(v_sem, 1)

        @block.sync
        def _(sync):
            sync.dma_start(out=w_tile[:], in_=wp[:]).then_inc(w_sem, 16)
            sync.wait_ge(v_sem, NCHUNK)
            sync.dma_start(out=op[0], in_=o_slots[SLOT[0]][:]).then_inc(s_sync, 16)
            sync.wait_ge(v_sem, 2 * NCHUNK + 4)
            sync.dma_start(
                out=op[2][:, :4 * RPC * W],
                in_=o_slots[SLOT[2]][:, :4 * RPC * W],
            ).then_inc(s_sync, 16)
            sync.wait_ge(v_sem, 3 * NCHUNK)
            sync.dma_start(
                out=op[2][:, 4 * RPC * W:],
                in_=o_slots[SLOT[2]][:, 4 * RPC * W:],
            ).then_inc(s_sync, 16)
            sync.wait_ge(s_sync, 48)

        @block.scalar
        def _(scalar):
            scalar.wait_ge(v_sem, 2 * NCHUNK)
            scalar.dma_start(out=op[1], in_=o_slots[SLOT[1]][:]).then_inc(s_scalar, 16)
            # first 3 chunks of image 3 as soon as they are copied
            scalar.wait_ge(v_sem, 3 * NCHUNK + 3)
            scalar.dma_start(
                out=op[3][:, :3 * RPC * W],
                in_=o_slots[SLOT[3]][:, :3 * RPC * W],
            ).then_inc(s_scalar, 16)
            scalar.wait_ge(s_scalar, 32)

    return nc


_NC_CACHE = None


def _get_nc():
    global _NC_CACHE
    if _NC_CACHE is None:
        _NC_CACHE = _build_nc()
    return _NC_CACHE


def _prep_weights(weights):
    """(32, 2048) grouped-sparse -> 16 [ic=128, oc=32] fp16 lhsT blocks."""
    w16 = np.zeros((C, 16 * 32), np.float32)
    for g, taps in enumerate(CFG):
        blk = np.asarray(weights[:, g * 512:(g + 1) * 512], np.float32)
        blk = blk.reshape(32, C, 4)  # [oc_in_group, ic, tap_j]
        for j in range(4):
            idx = g * 4 + j
            w16[:, idx * 32:(idx + 1) * 32] = blk[:, :, j].T
    return np.ascontiguousarray(w16.astype(np.float16))


def _prep_x(x):
    """(32,128,56,56) f32 -> per-core channel-major padded fp16 shards."""
    xpad = np.zeros((B, C, HP, WP), np.float16)
    xpad[:, :, 1:H + 1, 1:W + 1] = x.astype(np.float16)
    xs = xpad.reshape(NCORES, BPC, C, XF)
    # (core, b, c, f) -> (core, c, b*f)
    xs = np.ascontiguousarray(xs.transpose(0, 2, 1, 3)).reshape(NCORES, C, BPC * XF)
    return xs


def kernel(x, weights):
    x = np.asarray(x, np.float32)
    weights = np.asarray(weights, np.float32)

    xs = _prep_x(x)
    wflat = _prep_weights(weights)

    nc = _get_nc()
    in_maps = [{"x": xs[i], "w": wflat} for i in range(NCORES)]
    res = run_bass_kernel_spmd(nc, in_maps, core_ids=list(range(NCORES)))
    return np.concatenate(
        [res.results[i]["out"].reshape(BPC, C, H, W) for i in range(NCORES)],
        axis=0,
    )
